# revision 7
# baseline (speedup 1.0000x reference)
"""AWQ W4A16-style quantized linear (nn_AWQLinear) on 8 Trainium2 NeuronCores.

y[m,n] = sum_k x[m,k] * ((wq[n,k]*scales[n,g(k)] + zeros[n,g(k)]) / cs[k]) + bias[n]

Column-parallel over out_features (8 cores, N_shard = 1376/core).

Host (layout only): qweight transposed to byte-rows [K/2, N]; x transposed
with a k-permutation so each 128-row k-tile pair shares one byte-row block
(64 even-k + 64 odd-k rows per group); scales replicated to the per-k-row
pattern the dequant needs (fp16 round, same as a device-side cast).

Device per pair b (2 k-tiles):
  lo8 = (qb.u16 & 0x0f0f).u8              DVE   fused 16-bit bitwise, 2x mode
  hi8 = ((qb.u16 >> 4) & 0x0f0f).u8       DVE
  wlo = lo8 * srep -> f16                 DVE   tensor_tensor
  whi = hi8 * srep -> f16                 GPSIMD tensor_tensor
  12 matmuls [x16-tile slices]^T @ w-chunk -> 6 psum accumulators (fp16, 1 col/cyc)
  2 group-sum matmuls (0/1 pattern lhsT) -> S psum
Tail: zeros term as S16^T @ zerosT16 matmul; bias via 1-partition ones matmul;
ACT drains psum -> sbuf -> DMA out. The zeros/bias contributions never touch
per-element vector work.
"""
import numpy as np

import concourse.bacc as bacc
import concourse.mybir as mybir
from concourse import tile
from concourse.bass_utils import run_bass_kernel_spmd

IN_F = 4096          # K
OUT_F = 11008        # N
M_TOK = 256          # M
NCORES = 8
NSH = OUT_F // NCORES   # 1376
NPAIR = IN_F // 256     # 16 byte-row blocks of 128 rows (each -> 2 k-tiles)
CHUNKS = [(0, 512), (512, 512), (1024, NSH - 1024)]

F32, F16, U8, U16 = mybir.dt.float32, mybir.dt.float16, mybir.dt.uint8, mybir.dt.uint16


def _build_nc():
    nc = bacc.Bacc("TRN2", target_bir_lowering=False, debug=False,
                   num_devices=NCORES)

    xT_d = nc.dram_tensor("xT", [128, 32 * M_TOK], F32, kind="ExternalInput")
    csT_d = nc.dram_tensor("csT", [128, 32], F32, kind="ExternalInput")
    qwT_d = nc.dram_tensor("qwT", [IN_F // 2, NSH], U8, kind="ExternalInput")
    srep_d = nc.dram_tensor("srep", [NPAIR * 128, NSH], F16, kind="ExternalInput")
    zrT_d = nc.dram_tensor("zerosT", [32, NSH], F32, kind="ExternalInput")
    bias_d = nc.dram_tensor("bias", [1, NSH], F32, kind="ExternalInput")
    gpat_d = nc.dram_tensor("gpat", [128, NPAIR * 32], F16, kind="ExternalInput")
    y_d = nc.dram_tensor("y", [M_TOK, NSH], F32, kind="ExternalOutput")

    A = mybir.AluOpType

    with tile.TileContext(nc) as tc:
        with (
            tc.tile_pool(name="const", bufs=1) as cpool,
            tc.tile_pool(name="x16", bufs=1) as x16pool,
            tc.tile_pool(name="qb", bufs=4) as qbpool,
            tc.tile_pool(name="srep", bufs=4) as sreppool,
            tc.tile_pool(name="nib", bufs=4) as nibpool,
            tc.tile_pool(name="w", bufs=4) as wpool,
            tc.tile_pool(name="yout", bufs=2) as ypool,
            tc.tile_pool(name="ps", bufs=1, space="PSUM") as pspool,
        ):
            # ---- constants / small tensors ----
            csT = cpool.tile([128, 32], F32)
            nc.scalar.dma_start(csT[:], csT_d[:])
            rcs = cpool.tile([128, 32], F32)
            nc.vector.reciprocal(rcs[:], csT[:])

            # zeros^T with bias appended as row 32 -> one augmented matmul
            zrT32 = cpool.tile([32, NSH], F32)
            nc.scalar.dma_start(zrT32[:], zrT_d[:])
            zT16 = cpool.tile([33, NSH], F16)
            nc.scalar.copy(zT16[:32, :], zrT32[:])
            b32 = cpool.tile([1, NSH], F32)
            nc.scalar.dma_start(b32[:], bias_d[:])
            nc.scalar.copy(zT16[32:33, :], b32[:])

            gpat = cpool.tile([128, NPAIR * 32], F16)
            nc.scalar.dma_start(gpat[:], gpat_d[:])

            # ---- x pipeline: one big load, per-tile convert ----
            x32b = cpool.tile([128, 32 * M_TOK], F32)
            nc.scalar.dma_start(x32b[:], xT_d[:])
            x16 = []
            for t in range(32):
                xt = x16pool.tile([128, M_TOK], F16, tag=f"x16_{t}",
                                  name=f"x16_{t}")
                nc.scalar.mul(xt[:], x32b[:, t * M_TOK:(t + 1) * M_TOK],
                              rcs[:, t:t + 1])
                x16.append(xt)

            # ---- psum accumulators ----
            y_ps = [[pspool.tile([128, w], F32, tag=f"yps_{m}_{ci}",
                                 name=f"yps_{m}_{ci}")
                     for ci, (_, w) in enumerate(CHUNKS)] for m in range(2)]
            S_ps = pspool.tile([32, M_TOK], F32, tag="S_ps")

            # ---- main loop over byte-row blocks ----
            for b in range(NPAIR):
                qb = qbpool.tile([128, NSH], U8, tag="qb", name=f"qb_{b}")
                nc.sync.dma_start(qb[:], qwT_d[b * 128:(b + 1) * 128, :])

                srep = sreppool.tile([128, NSH], F16, tag="srep",
                                     name=f"srep_{b}")
                nc.sync.dma_start(srep[:], srep_d[b * 128:(b + 1) * 128, :])

                lo8 = nibpool.tile([128, NSH], U8, tag="lo8", name=f"lo8_{b}")
                nc.vector.tensor_scalar(lo8[:].bitcast(U16), in0=qb[:].bitcast(U16),
                                        scalar1=0x0F0F, scalar2=None,
                                        op0=A.bitwise_and)
                hi8 = nibpool.tile([128, NSH], U8, tag="hi8", name=f"hi8_{b}")
                nc.vector.tensor_scalar(hi8[:].bitcast(U16), in0=qb[:].bitcast(U16),
                                        scalar1=4, scalar2=0x0F0F,
                                        op0=A.logical_shift_right,
                                        op1=A.bitwise_and)

                wlo = wpool.tile([128, NSH], F16, tag="wlo", name=f"wlo_{b}")
                nc.vector.tensor_tensor(wlo[:], lo8[:], srep[:], A.mult)
                whi = wpool.tile([128, NSH], F16, tag="whi", name=f"whi_{b}")
                # split hi-mults ~3:1 GPSIMD:DVE to balance engine load
                eng = nc.vector if b % 4 == 3 else nc.gpsimd
                eng.tensor_tensor(whi[:], hi8[:], srep[:], A.mult)

                # group-sum matmuls (S[g, m] += sum_k x'[k, m] per group)
                nc.tensor.matmul(S_ps[:], gpat[:, b * 32:(b + 1) * 32],
                                 x16[2 * b][:], start=(b == 0), stop=False)
                nc.tensor.matmul(S_ps[:], gpat[:, b * 32:(b + 1) * 32],
                                 x16[2 * b + 1][:], start=False,
                                 stop=(b == NPAIR - 1))

                # main matmuls
                for w, xt in ((wlo, x16[2 * b]), (whi, x16[2 * b + 1])):
                    for m in range(2):
                        for ci, (c0, cw) in enumerate(CHUNKS):
                            nc.tensor.matmul(
                                y_ps[m][ci][:],
                                xt[:, m * 128:(m + 1) * 128],
                                w[:, c0:c0 + cw],
                                start=(b == 0 and w is wlo),
                                stop=False,
                            )

            # ---- tail: zeros+bias term (augmented matmul), drain ----
            S16 = cpool.tile([33, M_TOK], F16)
            nc.scalar.copy(S16[:32, :], S_ps[:])
            nc.vector.memset(S16[32:33, :], 1.0)
            for m in range(2):
                for ci, (c0, cw) in enumerate(CHUNKS):
                    nc.tensor.matmul(y_ps[m][ci][:],
                                     S16[:, m * 128:(m + 1) * 128],
                                     zT16[:, c0:c0 + cw],
                                     start=False, stop=True)
                    ysb = ypool.tile([128, cw], F32, tag=f"ysb_{ci}",
                                     name=f"ysb_{m}_{ci}")
                    nc.scalar.copy(ysb[:], y_ps[m][ci][:])
                    nc.scalar.dma_start(y_d[m * 128:(m + 1) * 128, c0:c0 + cw],
                                        ysb[:])
    nc.compile()
    return nc


def _host_prep(x, qweight, scales, zeros, channel_scales, bias):
    x2 = np.asarray(x, dtype=np.float32).reshape(M_TOK, IN_F)
    qw = np.asarray(qweight)
    if qw.dtype != np.uint8:
        qw = qw.astype(np.uint8)
    qwT = np.ascontiguousarray(qw.T)                      # [K/2, N]

    q = np.arange(128)
    perm = np.empty(IN_F, np.int64)
    for b in range(NPAIR):
        perm[(2 * b) * 128 + q] = 256 * b + 2 * q
        perm[(2 * b + 1) * 128 + q] = 256 * b + 2 * q + 1

    xT_perm = x2.T[perm]                                  # [K, M]
    # big-tile layout: partition q, free (t, m)
    xT_b = np.ascontiguousarray(
        xT_perm.reshape(32, 128, M_TOK).transpose(1, 0, 2).reshape(128, 32 * M_TOK))
    cs_perm = np.asarray(channel_scales, np.float32)[perm]
    csT = np.ascontiguousarray(cs_perm.reshape(32, 128).T)  # [128, 32]

    scalesT = np.asarray(scales, np.float32).T            # [32, N]
    # replicated per-k-row scale pattern, fp16 (same rounding a device cast
    # would apply): srep[b*128 + p, n] = scales[n, 2b + p//64]
    srep = np.empty((NPAIR * 128, OUT_F), np.float16)
    for b in range(NPAIR):
        srep[b * 128:b * 128 + 64] = scalesT[2 * b].astype(np.float16)
        srep[b * 128 + 64:(b + 1) * 128] = scalesT[2 * b + 1].astype(np.float16)

    zerosT = np.ascontiguousarray(np.asarray(zeros, np.float32).T)
    bias_f = np.asarray(bias, np.float32)

    gpat = np.zeros((128, NPAIR * 32), np.float16)
    for b in range(NPAIR):
        gpat[0:64, b * 32 + 2 * b] = 1.0
        gpat[64:128, b * 32 + 2 * b + 1] = 1.0

    return xT_b, csT, qwT, srep, zerosT, bias_f, gpat


def make_in_maps(x, qweight, scales, zeros, channel_scales, bias):
    xT_b, csT, qwT, srep, zerosT, bias_f, gpat = _host_prep(
        x, qweight, scales, zeros, channel_scales, bias)
    in_maps = []
    for c in range(NCORES):
        sl = slice(c * NSH, (c + 1) * NSH)
        in_maps.append({
            "xT": xT_b,
            "csT": csT,
            "qwT": np.ascontiguousarray(qwT[:, sl]),
            "srep": np.ascontiguousarray(srep[:, sl]),
            "zerosT": np.ascontiguousarray(zerosT[:, sl]),
            "bias": np.ascontiguousarray(bias_f[sl]).reshape(1, NSH),
            "gpat": gpat,
        })
    return in_maps


_NC_CACHE = {}


def get_nc():
    if "nc" not in _NC_CACHE:
        _NC_CACHE["nc"] = _build_nc()
    return _NC_CACHE["nc"]


def kernel(x, qweight, scales, zeros, channel_scales, bias):
    in_maps = make_in_maps(x, qweight, scales, zeros, channel_scales, bias)
    nc = get_nc()
    res = run_bass_kernel_spmd(nc, in_maps, core_ids=list(range(NCORES)))
    y = np.concatenate([res.results[c]["y"] for c in range(NCORES)], axis=1)
    return y.reshape(1, M_TOK, OUT_F).astype(np.float32)
